# revision 35
# baseline (speedup 1.0000x reference)
"""Trainium2 Bass kernel for nn_LLPKTMultiType (LLPKT knowledge tracing).

Strategy: pure data parallel, 4 samples per core x 8 cores.

The 850-step erase-add recurrence runs as DVE TensorTensorScan chunks
(partitions = 2 samples x 64 d, free = 50 concepts x 86 positions).
DVE is the bottleneck engine, so everything else is pushed off it:
  - dense phase (gathers/softmax/gates) uses PE + Scalar + Pool only;
    psum drains are Scalar activations, including direct strided writes
    of the qa-gate columns.
  - readouts: per-chunk product tensors (w * M) on DVE/Pool, then
    per-step sums via Scalar activation(Copy, accum_out=...).
  - samples 2-3's dense phase is issued interleaved with samples 0-1's
    scan chunks so it hides under the DVE scans.
"""

import os
import sys

import numpy as np

sys.path.insert(0, "/opt/trn_rl_repo")

B, S, L = 32, 50, 16
C, D = 50, 64
NQ, NL, NU = 10000, 2000, 5000
QV = NQ + NL + 1          # 12001
QAV = 2 * NQ + 1          # 20001
H4 = 4 * D                # 256
EPS = 1e-5

BL = 4                    # samples per core
NCORES = 8
NS = S * (L + 1)          # 850 flat update steps per sample
TC = 5                    # outer timesteps per scan chunk
SC = S // TC              # 10 chunks
CH = 17 * TC              # 85 update positions per chunk
LROWS = S * L             # 800 real lecture rows per sample
LPAD = 896                # padded to 7*128
QPAD = 128

_BUILT = None


def _build():
    import concourse.bass as bass
    import concourse.bacc as bacc
    import concourse.mybir as mybir
    import concourse.tile as tile
    from concourse.masks import make_identity

    f32 = mybir.dt.float32
    f16 = mybir.dt.float16
    i32 = mybir.dt.int32
    AX = mybir.AxisListType
    OP = mybir.AluOpType
    AF = mybir.ActivationFunctionType

    nc = bacc.Bacc("TRN2", target_bir_lowering=False, debug=False,
                   num_devices=NCORES)

    din = lambda n, sh, dt=f32: nc.dram_tensor(n, sh, dt, kind="ExternalInput").ap()
    idx_all = din("idx_all", [BL, 10, QPAD], i32)
    q_embed = din("q_embed", [QV, D])
    qa_embed = din("qa_embed", [QAV, D])
    key = din("key", [C, D])
    M0 = din("M0", [C, D])
    W_ea = din("W_ea", [D, 2 * D])               # W_e | W_a packed
    W0 = din("W0", [H4, H4])
    W1 = din("W1", [H4, H4])
    Wout = din("Wout", [H4])
    biases = din("biases", [2 * H4 + 2 * D])     # b0 | b1 | b_e | b_a
    gamma_beta = din("gamma_beta", [2 * H4])
    b_out = din("b_out", [1])
    preds = nc.dram_tensor("preds", [BL, S], f32, kind="ExternalOutput").ap()

    with tile.TileContext(nc) as tc:
        with (
            tc.tile_pool(name="persist", bufs=1) as pp,
            tc.tile_pool(name="xt", bufs=2) as xp,
            tc.tile_pool(name="gs", bufs=2) as gsp,
            tc.tile_pool(name="gather", bufs=6) as gp,
            tc.tile_pool(name="cw", bufs=3) as cw,
            tc.tile_pool(name="cuv", bufs=3) as cuv,
            tc.tile_pool(name="cm", bufs=2) as cm,
            tc.tile_pool(name="prod", bufs=2) as prp,
            tc.tile_pool(name="psum", bufs=6, space="PSUM") as psp,
            tc.tile_pool(name="psum2", bufs=2, space="PSUM") as psp2,
            tc.tile_pool(name="dram", bufs=1, space="DRAM") as dp,
        ):
            # ---------------- constants / weights ----------------
            ident = pp.tile([128, 128], f32, tag="ident")
            make_identity(nc, ident[:])

            KT = pp.tile([D, C], f32, tag="KT")                    # [d, c]
            nc.sync.dma_start(KT[:], key.rearrange("c d -> d c"))
            Wea_sb = pp.tile([D, 2 * D], f32, tag="Wea")
            nc.sync.dma_start(Wea_sb[:], W_ea)
            W01 = pp.tile([128, 4, H4], f32, tag="W01")
            Wout_rep = pp.tile([128, H4], f32, tag="Woutr")
            bias_rep = pp.tile([128, 2 * H4 + 2 * D], f32, tag="biasr")
            gb_rep = pp.tile([S, 2 * H4], f32, tag="gbr")
            bout_rep = pp.tile([128, 1], f32, tag="boutr")
            M0Tf = pp.tile([128, C], f32, tag="M0Tf")
            M0T = pp.tile([128, C], f16, tag="M0T")
            # per-partition gate biases applied at psum drain:
            # rows 0:D = b_e (erase), rows D:2D = b_a (add)
            bias_ea = pp.tile([128, 1], f32, tag="biasea")
            nc.sync.dma_start(bias_ea[0:D, :], biases[2 * H4:2 * H4 + D][:, None])
            nc.sync.dma_start(bias_ea[D:2 * D, :],
                              biases[2 * H4 + D:2 * H4 + 2 * D][:, None])

            E = [pp.tile([128, NS], f16, tag=f"E{p}", name=f"E{p}") for p in range(2)]
            A = [pp.tile([128, NS], f16, tag=f"A{p}", name=f"A{p}") for p in range(2)]
            lr = [pp.tile([128, S], f32, tag=f"lr{p}", name=f"lr{p}") for p in range(2)]
            qr = [pp.tile([128, S], f32, tag=f"qr{p}", name=f"qr{p}") for p in range(2)]
            q_raw = [pp.tile([128, D], f32, tag=f"qraw{b}", name=f"qraw{b}") for b in range(BL)]
            le_raw = [pp.tile([128, D], f32, tag=f"leraw{b}", name=f"leraw{b}") for b in range(BL)]
            wT_sb = [pp.tile([C, NS], f16, tag=f"wT{b}", name=f"wT{b}") for b in range(BL)]
            # w stored per sample as [chunk, c, s] so each broadcast
            # descriptor is one contiguous 8.5KB burst per partition
            w_dram = [dp.tile([SC, C, CH], f16,
                              tag=f"wdram{b}", name=f"wdram{b}")
                      for b in range(BL)]

            def psum_t():
                return psp.tile([128, 128], f32, space="PSUM", tag="tp",
                                name="tp")

            # ---------------- dense phase building blocks ----------------
            def load_idx(b):
                it = gp.tile([128, 10], i32, tag="idx")
                nc.sync.dma_start(it[:], idx_all[b].rearrange("j r -> r j"))
                return it

            def gather_chunk(it, j, table, dst=None):
                g = dst if dst is not None else gp.tile([128, D], f32, tag="graw",
                                                        name="graw")
                nc.gpsimd.indirect_dma_start(
                    out=g[:], out_offset=None, in_=table,
                    in_offset=bass.IndirectOffsetOnAxis(ap=it[:, j:j + 1], axis=0))
                return g

            def xT_of(g, tag):
                ps = psum_t()[0:D, :]
                nc.tensor.transpose(out=ps, in_=g[:], identity=ident[:])
                xT = xp.tile([D, 128], f32, tag=tag)
                nc.scalar.activation(xT[:], ps, AF.Copy)
                return xT

            def corr_T(xT):
                """softmax(x @ K^T) transposed -> [C, 128] psum (f32).

                Logits are tiny (|x| ~ .1) so the max-subtraction is skipped.
                All drains on Scalar; only the reciprocal is DVE.
                """
                psc = psum_t()[:, 0:C]
                nc.tensor.matmul(psc, lhsT=xT[:], rhs=KT[:],
                                 start=True, stop=True)
                wexp = gp.tile([128, C], f32, tag="wexp")
                se = gp.tile([128, 1], f32, tag="se")
                nc.scalar.activation(wexp[:], psc, AF.Exp,
                                     accum_out=se[:, :1])
                rse = gp.tile([128, 1], f32, tag="rse")
                nc.vector.reciprocal(rse[:], se[:])
                wsb = gp.tile([128, C], f32, tag="wsb")
                nc.scalar.activation(wsb[:], wexp[:], AF.Copy,
                                     scale=rse[:, 0:1])
                pst = psum_t()[0:C, :]
                nc.tensor.transpose(out=pst, in_=wsb[:],
                                    identity=ident[:])
                return pst

            def gates_psum(xT):
                """raw gate logits, transposed: [2D, n] psum."""
                psg = psum_t()
                nc.tensor.matmul(psg[:, 0:2 * D], lhsT=xT[:],
                                 rhs=Wea_sb[:], start=True, stop=True)
                gs = gsp.tile([128, 2 * D], f32, tag="gsd")
                nc.scalar.activation(gs[:], psg[:, 0:2 * D], AF.Copy)
                pst = psum_t()
                nc.tensor.transpose(out=pst[:], in_=gs[:], identity=ident[:])
                return pst

            def ea_view(b):
                pr, half = b // 2, 64 * (b % 2)
                E3 = E[pr][half:half + D, :].rearrange(
                    "p (t k) -> p t k", k=17)
                A3 = A[pr][half:half + D, :].rearrange(
                    "p (t k) -> p t k", k=17)
                return E3, A3

            def gather_part(b, it, j, gts):
                """issue the indirect gather for block j ahead of its use."""
                if j == "q":
                    gts[(b, "q")] = gather_chunk(it, 7, q_embed, dst=q_raw[b])
                    gather_chunk(it, 9, q_embed, dst=le_raw[b])
                    gts[(b, "qa")] = gather_chunk(it, 8, qa_embed)
                    return
                g = gp.tile([128, D], f32, tag=f"g{b % 2}_{j}",
                            name=f"g{b % 2}_{j}", bufs=2)
                gather_chunk(it, j, q_embed, dst=g)
                gts[(b, j)] = g

            def corr_part(b, j, gts, xts):
                """softmax-weights part (Exp + Copy activations only)."""
                wT3 = wT_sb[b][:].rearrange("c (t k) -> c t k", k=17)
                if j == "q":
                    xT = xT_of(gts[(b, "q")], f"xTq{b % 2}")
                    pstq = corr_T(xT)
                    nc.scalar.activation(wT3[:, 0:S, 16], pstq[:, 0:S],
                                         AF.Copy)
                    xts[(b, "qa")] = xT_of(gts[(b, "qa")], f"xTqa{b % 2}")
                    return
                t0 = 8 * j
                tcnt = min(8, S - t0)
                xT = xT_of(gts[(b, j)], f"xT{b % 2}_{j}")
                xts[(b, j)] = xT
                pstc = corr_T(xT)
                nc.scalar.activation(
                    wT3[:, t0:t0 + tcnt, 0:16],
                    pstc[:, 0:tcnt * 16].rearrange("c (t k) -> c t k", k=16),
                    AF.Copy)

            def gate_part(b, j, xts):
                """erase/add gates (Sigmoid + Tanh activations)."""
                E3, A3 = ea_view(b)
                if j == "qa":
                    pst = gates_psum(xts[(b, "qa")])
                    nc.scalar.activation(E3[:, 0:S, 16], pst[0:D, 0:S],
                                         AF.Sigmoid, bias=bias_ea[0:D, 0:1])
                    nc.scalar.activation(A3[:, 0:S, 16], pst[D:2 * D, 0:S],
                                         AF.Tanh, bias=bias_ea[D:2 * D, 0:1])
                    return
                t0 = 8 * j
                tcnt = min(8, S - t0)
                pst = gates_psum(xts[(b, j)])
                pst3 = pst[:].rearrange("p (t k) -> p t k", k=16)
                nc.scalar.activation(E3[:, t0:t0 + tcnt, 0:16],
                                     pst3[0:D, 0:tcnt, :],
                                     AF.Sigmoid, bias=bias_ea[0:D, 0:1])
                nc.scalar.activation(A3[:, t0:t0 + tcnt, 0:16],
                                     pst3[D:2 * D, 0:tcnt, :],
                                     AF.Tanh, bias=bias_ea[D:2 * D, 0:1])

            def store_w(b, ch):
                nc.sync.dma_start(w_dram[b][ch],
                                  wT_sb[b][:, CH * ch:CH * (ch + 1)])

            def corr_store(b, j, gts, xts):
                corr_part(b, j, gts, xts)
                lo = 0 if j == 0 else (8 * (j - 1) + 3) // 5 + 1
                hi = min(SC, (8 * j + 3) // 5 + 1)
                for ch in range(lo, hi):
                    store_w(b, ch)

            def dense_pair_plan(b0, b1, split):
                """Dense work for a sample pair as (pre, rest) closure lists.

                split=False: everything in `pre`, with all corr parts (Exp
                table) before all gate parts (Sigmoid table) so the act
                table switches only twice.
                split=True: `pre` holds just what scan chunks 0-2 need
                (q/qa + lecture blocks j0, j1); `rest` has one closure per
                remaining lecture block, each self-contained (corr + gates
                + w stores), for interleaving under the scan chunks.
                """
                its = {}
                gts = {}
                xts = {}
                jpre = 2 if split else 7

                def _start(b):
                    its[b] = load_idx(b)
                pre = []
                for b in (b0, b1):
                    pre.append(lambda b=b: _start(b))
                    pre.append(lambda b=b: gather_part(b, its[b], "q", gts))
                for j in range(jpre):
                    for b in (b0, b1):
                        pre.append(
                            lambda b=b, j=j: gather_part(b, its[b], j, gts))
                for b in (b0, b1):
                    pre.append(lambda b=b: corr_part(b, "q", gts, xts))
                for j in range(jpre):
                    for b in (b0, b1):
                        pre.append(
                            lambda b=b, j=j: corr_store(b, j, gts, xts))
                for b in (b0, b1):
                    pre.append(lambda b=b: gate_part(b, "qa", xts))
                for j in range(jpre):
                    for b in (b0, b1):
                        pre.append(lambda b=b, j=j: gate_part(b, j, xts))
                rest = []
                for j in range(jpre, 7):
                    def _grp(j=j):
                        for b in (b0, b1):
                            gather_part(b, its[b], j, gts)
                        for b in (b0, b1):
                            corr_store(b, j, gts, xts)
                        for b in (b0, b1):
                            gate_part(b, j, xts)
                    rest.append(_grp)
                return pre, rest

            # ---------------- scan phase ----------------
            def scan_prep(pr, ch):
                s0 = CH * ch
                wb = cw.tile([128, C * CH], f16, tag="wb")
                wb3 = wb[:].rearrange("p (c s) -> p c s", s=CH)
                for bi, b in enumerate((2 * pr, 2 * pr + 1)):
                    dst3 = wb[D * bi:D * bi + D, :].rearrange(
                        "p (c s) -> p c s", s=CH)
                    srcb = w_dram[b][ch][None, :, :].to_broadcast([D, C, CH])
                    nc.sync.dma_start(dst3[:, :, :], srcb[:, :, :])
                u = cuv.tile([128, C * (CH + 1)], f16, tag="u")
                v = cuv.tile([128, C * (CH + 1)], f16, tag="v")
                u3 = u[:].rearrange("p (c s) -> p c s", s=CH + 1)
                v3 = v[:].rearrange("p (c s) -> p c s", s=CH + 1)
                Eb = E[pr][:, s0:s0 + CH][:, None, :].to_broadcast([128, C, CH])
                Ab = A[pr][:, s0:s0 + CH][:, None, :].to_broadcast([128, C, CH])
                nc.vector.tensor_tensor(u3[:, :, 1:], wb3[:, :, :],
                                        Eb[:, :, :], op=OP.mult)
                nc.scalar.activation(u3[:, :, 1:], u3[:, :, 1:], AF.Copy,
                                     bias=1.0, scale=-1.0)
                nc.gpsimd.memset(u3[:, :, 0:1], 0.0)
                nc.vector.tensor_tensor(v3[:, :, 1:], wb3[:, :, :],
                                        Ab[:, :, :], op=OP.mult)
                if ch == 0:
                    nc.gpsimd.tensor_copy(v3[:, :, 0:1], M0T[:][:, :, None])
                return (wb, u, v, v3)

            def scan_compute(pr, ch, cur, nxt):
                """scan + fused product/accumulate readouts (DVE STTs)."""
                wb, u, v, _ = cur
                wb3 = wb[:].rearrange("p (c s) -> p c s", s=CH)
                Mt = cm.tile([128, C * (CH + 1)], f16, tag="Mt")
                Mt3 = Mt[:].rearrange("p (c s) -> p c s", s=CH + 1)
                nc.vector.tensor_tensor_scan(
                    Mt[:], u[:], v[:], 0.0, op0=OP.mult, op1=OP.add)
                if nxt is not None:
                    nc.vector.tensor_copy(nxt[3][:, :, 0:1],
                                          Mt3[:, :, CH:CH + 1])
                scr = prp.tile([128, C * 16], f16, tag="scr")
                scr3 = scr[:].rearrange("p (c k) -> p c k", k=16)
                scr2 = prp.tile([128, C], f16, tag="scr2")
                for tl in range(TC):
                    t = TC * ch + tl
                    sl = 17 * tl
                    nc.vector.scalar_tensor_tensor(
                        out=scr3, in0=wb3[:, :, sl:sl + 16], scalar=1.0,
                        op0=OP.mult, in1=Mt3[:, :, sl:sl + 16], op1=OP.mult,
                        accum_out=lr[pr][:, t:t + 1])
                    nc.vector.scalar_tensor_tensor(
                        out=scr2[:][:, :, None],
                        in0=wb3[:, :, sl + 16:sl + 17],
                        scalar=1.0, op0=OP.mult, in1=Mt3[:, :, sl:sl + 1],
                        op1=OP.mult, accum_out=qr[pr][:, t:t + 1])

            def scan_chain(pr, interleave=None, chunks=None, at=None,
                           drain_by=SC):
                """at: dict ch -> closures issued right after that chunk's
                compute. interleave closures are spread over chunks and
                fully drained by chunk `drain_by`-1."""
                if chunks is None:
                    chunks = [scan_prep(pr, 0), scan_prep(pr, 1)]
                for ch in range(SC):
                    nxt = chunks[ch + 1] if ch + 1 < SC else None
                    scan_compute(pr, ch, chunks[ch], nxt)
                    if interleave:
                        left = max(1, drain_by - ch)
                        n = max(1, (len(interleave) + left - 1) // left)
                        for _ in range(n):
                            if interleave:
                                interleave.pop(0)()
                    if at and ch in at:
                        for c in at[ch]:
                            c()
                    if ch + 2 < SC:
                        chunks.append(scan_prep(pr, ch + 2))

            # ---------------- readout: mastery -> LN -> MLP ----------------
            def readout_pr(pr, t0, t1):
                tn = t1 - t0
                rows = 2 * tn  # (b within pair, t) rows
                ms = pp.tile([S, 2 * H4], f32, tag=f"ms{pr}_{t0}",
                             name=f"ms{pr}_{t0}")[0:tn, :]
                for which, tsrc in ((0, qr[pr]), (2, lr[pr])):
                    pst = psum_t()[0:tn, :]
                    nc.tensor.transpose(out=pst, in_=tsrc[:, t0:t1],
                                        identity=ident[:])
                    for bh in range(2):
                        nc.scalar.activation(
                            ms[:, bh * H4 + which * D:bh * H4 + (which + 1) * D],
                            pst[:, bh * D:(bh + 1) * D], AF.Copy)
                for bh in range(2):
                    b = 2 * pr + bh
                    nc.sync.dma_start(ms[:, bh * H4 + D:bh * H4 + 2 * D],
                                      q_raw[b][t0:t1, :])
                    nc.sync.dma_start(ms[:, bh * H4 + 3 * D:bh * H4 + 4 * D],
                                      le_raw[b][t0:t1, :])
                # LayerNorm via E[x], E[x^2] on Scalar accumulators, then
                # x*scale + bias with scale = rstd*gamma folded per row.
                ms3 = ms.rearrange("p (b f) -> p b f", f=H4)
                mean = pp.tile([S, 2], f32, tag=f"mean{pr}_{t0}", name=f"mean{pr}_{t0}")[0:tn, :]
                msq = pp.tile([S, 2], f32, tag=f"msq{pr}_{t0}", name=f"msq{pr}_{t0}")[0:tn, :]
                scr5 = pp.tile([S, H4], f32, tag=f"scr5_{pr}_{t0}", name=f"scr5_{pr}_{t0}")[0:tn, :]
                for bh in range(2):
                    sl = slice(bh * H4, (bh + 1) * H4)
                    nc.scalar.activation(scr5, ms[:, sl], AF.Copy,
                                         scale=1.0 / H4,
                                         accum_out=mean[:, bh:bh + 1])
                    nc.scalar.activation(scr5, ms[:, sl], AF.Square,
                                         scale=1.0 / 16.0,
                                         accum_out=msq[:, bh:bh + 1])
                var = pp.tile([S, 2], f32, tag=f"var{pr}_{t0}", name=f"var{pr}_{t0}")[0:tn, :]
                nc.vector.tensor_tensor(var, mean, mean, op=OP.mult)
                nc.vector.tensor_tensor(var, msq, var, op=OP.subtract)
                nc.vector.tensor_scalar(var, var, 1.0, EPS,
                                        op0=OP.mult, op1=OP.add)
                sd = pp.tile([S, 2], f32, tag=f"sd{pr}_{t0}", name=f"sd{pr}_{t0}")[0:tn, :]
                nc.scalar.activation(sd, var, AF.Sqrt)
                rsd = pp.tile([S, 2], f32, tag=f"rsd{pr}_{t0}", name=f"rsd{pr}_{t0}")[0:tn, :]
                nc.vector.reciprocal(rsd, sd)
                # scale = rstd * gamma, bias = beta - mean * scale (per row)
                gmb = gb_rep[0:tn, 0:H4][:, None, :].to_broadcast([tn, 2, H4])
                btb = gb_rep[0:tn, H4:2 * H4][:, None, :].to_broadcast([tn, 2, H4])
                scl = pp.tile([S, 2 * H4], f32, tag=f"scl{pr}_{t0}", name=f"scl{pr}_{t0}")[0:tn, :]
                scl3 = scl.rearrange("p (b f) -> p b f", f=H4)
                bia = pp.tile([S, 2 * H4], f32, tag=f"bia{pr}_{t0}", name=f"bia{pr}_{t0}")[0:tn, :]
                bia3 = bia.rearrange("p (b f) -> p b f", f=H4)
                nc.vector.tensor_tensor(
                    scl3, gmb, rsd[:, :, None].to_broadcast([tn, 2, H4]),
                    op=OP.mult)
                nc.vector.tensor_tensor(
                    bia3, scl3, mean[:, :, None].to_broadcast([tn, 2, H4]),
                    op=OP.mult)
                nc.vector.tensor_tensor(bia3, btb, bia3, op=OP.subtract)
                nc.vector.tensor_tensor(ms3, ms3, scl3, op=OP.mult)
                nc.vector.tensor_tensor(ms3, ms3, bia3, op=OP.add)
                msT_lo = pp.tile([128, 2 * S], f32, tag=f"msTlo{pr}_{t0}", name=f"msTlo{pr}_{t0}")
                msT_hi = pp.tile([128, 2 * S], f32, tag=f"msThi{pr}_{t0}", name=f"msThi{pr}_{t0}")
                for bh in range(2):
                    b = 2 * pr + bh
                    for fh, dstT in ((0, msT_lo), (1, msT_hi)):
                        pst = psum_t()[:, 0:tn]
                        nc.tensor.transpose(
                            out=pst,
                            in_=ms[:, bh * H4 + fh * 128:bh * H4 + (fh + 1) * 128],
                            identity=ident[0:tn, 0:tn])
                        nc.scalar.activation(dstT[:, bh * tn:(bh + 1) * tn],
                                             pst, AF.Copy)

                ph = psp2.tile([2 * S, H4], f32, space="PSUM", tag="mlp",
                               name="mlp")[0:rows, :]
                nc.tensor.matmul(ph, lhsT=msT_lo[:, 0:rows], rhs=W01[:, 0, :],
                                 start=True, stop=False)
                nc.tensor.matmul(ph, lhsT=msT_hi[:, 0:rows], rhs=W01[:, 1, :],
                                 start=False, stop=True)
                h1 = pp.tile([2 * S, H4], f32, tag=f"h1_{pr}_{t0}", name=f"h1_{pr}_{t0}")[0:rows, :]
                nc.vector.tensor_tensor(h1, ph, bias_rep[0:rows, 0:H4],
                                        op=OP.add)
                nc.scalar.activation(h1, h1, AF.Relu)
                h1T = [pp.tile([128, 2 * S], f32, tag=f"h1T{fh}_{pr}_{t0}", name=f"h1T{fh}_{pr}_{t0}")
                       for fh in range(2)]
                for fh in range(2):
                    pst = psum_t()[:, 0:rows]
                    nc.tensor.transpose(out=pst,
                                        in_=h1[:, fh * 128:(fh + 1) * 128],
                                        identity=ident[0:rows, 0:rows])
                    nc.scalar.activation(h1T[fh][:, 0:rows], pst, AF.Copy)
                ph2 = psp2.tile([2 * S, H4], f32, space="PSUM", tag="mlp",
                                name="mlp")[0:rows, :]
                nc.tensor.matmul(ph2, lhsT=h1T[0][:, 0:rows], rhs=W01[:, 2, :],
                                 start=True, stop=False)
                nc.tensor.matmul(ph2, lhsT=h1T[1][:, 0:rows], rhs=W01[:, 3, :],
                                 start=False, stop=True)
                h2 = pp.tile([2 * S, H4], f32, tag=f"h2_{pr}_{t0}", name=f"h2_{pr}_{t0}")[0:rows, :]
                nc.vector.tensor_tensor(h2, ph2,
                                        bias_rep[0:rows, H4:2 * H4], op=OP.add)
                scr4 = pp.tile([2 * S, H4], f32, tag=f"scr4_{pr}_{t0}", name=f"scr4_{pr}_{t0}")[0:rows, :]
                logit = pp.tile([2 * S, 1], f32, tag=f"logit{pr}_{t0}", name=f"logit{pr}_{t0}")[0:rows, :]
                nc.vector.scalar_tensor_tensor(
                    out=scr4, in0=h2, scalar=1.0, op0=OP.mult,
                    in1=Wout_rep[0:rows, :], op1=OP.mult,
                    accum_out=logit[:, 0:1])
                psig = pp.tile([2 * S, 1], f32, tag=f"psig{pr}_{t0}", name=f"psig{pr}_{t0}")[0:rows, :]
                nc.scalar.activation(psig, logit, AF.Sigmoid,
                                     bias=bout_rep[0:rows, 0:1], scale=1.0)
                for bh in range(2):
                    nc.sync.dma_start(
                        preds[2 * pr + bh, t0:t1][:, None],
                        psig[bh * tn:(bh + 1) * tn, 0:1])

            def load_late_weights():
                nc.sync.dma_start(W01[:, 0, :], W0[0:128, :])
                nc.sync.dma_start(W01[:, 1, :], W0[128:256, :])
                nc.sync.dma_start(W01[:, 2, :], W1[0:128, :])
                nc.sync.dma_start(W01[:, 3, :], W1[128:256, :])
                nc.sync.dma_start(Wout_rep[:],
                                  Wout[None, :].to_broadcast([128, H4]))
                nc.sync.dma_start(
                    bias_rep[:],
                    biases[None, :].to_broadcast([128, 2 * H4 + 2 * D]))
                nc.sync.dma_start(
                    gb_rep[:], gamma_beta[None, :].to_broadcast([S, 2 * H4]))
                nc.sync.dma_start(bout_rep[:],
                                  b_out[None, :].to_broadcast([128, 1]))
                for bb in range(2):
                    nc.sync.dma_start(M0Tf[D * bb:D * bb + D, :],
                                      M0.rearrange("c d -> d c"))
                nc.scalar.activation(M0T[:], M0Tf[:], AF.Copy)

            # ---------------- program ----------------
            pre01, rest01 = dense_pair_plan(0, 1, split=True)
            for step in pre01:
                step()
            load_late_weights()
            pre23, _ = dense_pair_plan(2, 3, split=False)
            inter = rest01 + pre23
            chunks1 = []
            at0 = {8: [lambda: chunks1.append(scan_prep(1, 0))],
                   9: [lambda: chunks1.append(scan_prep(1, 1))]}
            scan_chain(0, interleave=inter, at=at0, drain_by=8)
            for step in inter:
                step()
            at1 = {0: [lambda: readout_pr(0, 0, 25)],
                   1: [lambda: readout_pr(0, 25, 50)],
                   6: [lambda: readout_pr(1, 0, 25)]}
            scan_chain(1, chunks=chunks1, at=at1)
            readout_pr(1, 25, 50)

    nc.compile()
    return nc


def _host_prepare(inputs):
    q_data = np.asarray(inputs["q_data"]).astype(np.int32)
    qa_data = np.asarray(inputs["qa_data"]).astype(np.int32)
    l_data = np.asarray(inputs["l_data"]).astype(np.int32)
    f = lambda k: np.ascontiguousarray(np.asarray(inputs[k]), dtype=np.float32)
    q_embed, qa_embed = f("q_embed"), f("qa_embed")
    key, M0 = f("key_matrix"), f("M0")
    W_ea = np.concatenate([f("W_e"), f("W_a")], axis=1)
    biases = np.concatenate([f("b0"), f("b1"), f("b_e"), f("b_a")])
    gamma_beta = np.concatenate([f("ln_gamma"), f("ln_beta")])
    W0, W1 = f("W0"), f("W1")
    Wout = f("W_out").reshape(-1)
    b_out = f("b_out").reshape(-1)

    in_maps = []
    for core in range(NCORES):
        bs = slice(core * BL, (core + 1) * BL)
        idx = np.zeros((BL, 10, QPAD), np.int32)
        idx[:, 0:7, :].reshape(BL, LPAD)[:, :LROWS] = \
            l_data[bs].reshape(BL, LROWS)
        idx[:, 7, :S] = q_data[bs]
        idx[:, 8, :S] = qa_data[bs]
        idx[:, 9, :S] = l_data[bs][:, :, L - 1]
        in_maps.append(dict(
            idx_all=np.ascontiguousarray(idx),
            q_embed=q_embed, qa_embed=qa_embed, key=key, M0=M0,
            W_ea=W_ea, W0=W0, W1=W1, Wout=Wout, biases=biases,
            gamma_beta=gamma_beta, b_out=b_out,
        ))
    return in_maps


def kernel(**inputs):
    global _BUILT
    if _BUILT is None:
        _BUILT = _build()
    nc = _BUILT
    from concourse import bass_utils
    in_maps = _host_prepare(inputs)
    res = bass_utils.run_bass_kernel_spmd(
        nc, in_maps, core_ids=list(range(NCORES)),
        trace=bool(int(os.environ.get("KERNEL_TRACE", "0"))))
    out = np.concatenate([r["preds"] for r in res.results], axis=0)
    kernel.last_results = res
    return out
